# revision 23
# baseline (speedup 1.0000x reference)
"""Trainium2 Bass kernel for nn_MetaBEVWithModalFusion.

Strategy (8 NeuronCores, SPMD, identical program, data-parallel over 512-token
query slices):
  - tokens: 4 blocks x 1024 block-tokens = 4096; core c owns block c//2,
    half c%2 (512 q tokens). All layouts are feature-major x_T [C, tokens]
    (matches the channels-first inputs), except v / MoE which are token-major.
  - Phase A (per core): 3 cross-attentions (q=my 512 meta tokens, k=full
    1024-token block) + dense soft-MoE. Attention uses transposed logits
    [k, q] (no max subtraction -- logits are tiny for this model family),
    exp on ACT, softmax denominators via ones-matmuls, a@v via col-packed
    4-head matmuls, deferred normalization with an fp32r ones-broadcast.
  - One AllGather (bf16, 256KB/rank) exchanges MoE outputs.
  - Phase B: full-sequence self-attention, q=my 512 tokens, k=all 4096.
  - bf16 matmul operands, fp32 PSUM accumulation + softmax statistics.
"""

import math
from contextlib import ExitStack

import ml_dtypes
import numpy as np

import concourse.bass as bass
import concourse.mybir as mybir
import concourse.tile as tile
from concourse.vector_clock import VectorClock, ScopedClock
from concourse.bass_utils import run_bass_kernel_spmd

F32 = mybir.dt.float32
F32R = mybir.dt.float32r
BF = mybir.dt.bfloat16
BF_NP = ml_dtypes.bfloat16
EXP = mybir.ActivationFunctionType.Exp

N_CORES = 8
E = 256
NH = 8
DH = 32
Q = 512  # q tokens per core


def _patched_drain(self, tick_clock, wait_clock):
    # This walrus build cannot encode >1 semaphore wait on the tail Drain
    # (NO_STRUCT); split the final-clock waits across SP NOPs issued before it.
    gc = tick_clock.global_clock
    n = len(gc)
    for p in range(n):
        if gc[p] > 0:
            sub = VectorClock([gc[i] if i == p else 0 for i in range(n)])
            nop = self.nc.sync.nop()
            wait_clock.add_sem_waits(nop.ins, ScopedClock({None: sub}))
    self.nc.sync.drain()
    self.nc.all_engine_barrier()
    popped = self.nc._tile_sem_poison_stack.pop()
    assert popped is self._sem_poison
    self.nc.clear_and_free_semaphores(list(self.sems.allocated().values()))
    self.nc.all_engine_barrier()


tile.TileContext._drain_and_barrier = _patched_drain


def _split_multi_waits(nc):
    """This walrus build encodes at most ONE sem wait per instruction; peel
    excess waits onto same-engine NoOps placed immediately before."""
    for fn in nc.m.functions:
        for bb in fn.blocks:
            new = []
            changed = False
            for inst in bb.instructions:
                si = inst.sync_info
                if si is not None and si.on_wait and len(si.on_wait) > 1:
                    changed = True
                    waits = list(si.on_wait)
                    for w in waits[:-1]:
                        nop = mybir.InstNoOp(
                            name=f"I-wsplit-{nc.next_id()}", ins=[], outs=[]
                        )
                        nop.engine = inst.engine
                        nop.sync_info = mybir.SyncInfo(on_wait=[w], on_update=[])
                        new.append(nop)
                    si.on_wait = [waits[-1]]
                new.append(inst)
            if changed:
                bb.instructions[:] = new


def _load_weight_pair(nc, pool, name, param, dtype, width):
    """DMA a [256, width] DRAM param into two [128, width] SBUF tiles."""
    ts = []
    for ic in range(2):
        t = pool.tile([128, width], dtype, tag=f"{name}{ic}", name=f"{name}{ic}")
        nc.sync.dma_start(out=t[:], in_=param[128 * ic : 128 * (ic + 1), :])
        ts.append(t)
    return ts


def _proj_fm(nc, pp, spool, name, wT, x_tiles, n_tok, bias=None, scale_mm=None):
    """Feature-major projection: out_T[oc] [128, n_tok] = (W @ x)_chunk + b.

    wT: 2 tiles [128(in chunk), 256(out)]; x_tiles: 2 tiles [128, n_tok].
    Returns two SBUF bf16 tiles [128, n_tok]."""
    outs = []
    nchunks = n_tok // 512
    for oc in range(2):
        o = spool.tile([128, n_tok], BF, tag=f"{name}{oc}", name=f"{name}{oc}")
        outs.append(o)
        for nck in range(nchunks):
            ps = pp.tile([128, 512], F32, tag="p512", name="proj_ps")
            for ic in range(2):
                nc.tensor.matmul(
                    ps[:],
                    lhsT=wT[ic][:, 128 * oc : 128 * (oc + 1)],
                    rhs=x_tiles[ic][:, 512 * nck : 512 * (nck + 1)],
                    start=(ic == 0),
                    stop=(ic == 1),
                )
            dst = o[:, 512 * nck : 512 * (nck + 1)]
            if bias is not None:
                nc.vector.tensor_scalar_add(dst, ps[:], bias[oc][:, 0:1])
            else:
                nc.vector.tensor_copy(out=dst, in_=ps[:])
    return outs


def _proj_tm(nc, pp, spool, name, wT, x_tiles, n_tok):
    """Token-major projection: v [128(tok chunk tc), 256] packed into one
    [128, (n_tok//128)*256] tile, token-chunk tc at cols [256*tc, 256*tc+256)."""
    tchunks = n_tok // 128
    v = spool.tile([128, 256 * tchunks], BF, tag=f"{name}", name=f"{name}")
    for tc in range(tchunks):
        ps = pp.tile([128, 256], F32, tag="p256", name="projv_ps")
        for ic in range(2):
            nc.tensor.matmul(
                ps[:],
                lhsT=x_tiles[ic][:, 128 * tc : 128 * (tc + 1)],
                rhs=wT[ic][:],
                start=(ic == 0),
                stop=(ic == 1),
            )
        nc.vector.tensor_copy(out=v[:, 256 * tc : 256 * (tc + 1)], in_=ps[:])
    return v


def _emit_attention(nc, Lp, osp, apool, qT, kT, v_sb, k_chunks, ones_b32, gtag):
    """Multi-head attention with q=512 (feature-major qT), k=k_chunks*128.

    qT/kT: 2 tiles [128, *] with heads 4g..4g+3 at partition strips 32h of
    chunk g. v_sb: token-major [128, 256*k_chunks]. Returns [oT0, oT1]
    (bf16 [128, 512], softmax-normalized, feature-major o)."""
    oTs = []
    for g in range(2):
        o_ps = osp.tile([128, 512], F32, tag="o", name="o_ps")
        s_ps = osp.tile([128, 512], F32, tag="s", name="s_ps")
        for c in range(k_chunks):
            L = Lp.tile([128, 2048], F32, tag="L", name="L")
            for h in range(4):
                nc.tensor.matmul(
                    L[:, 512 * h : 512 * (h + 1)],
                    lhsT=kT[g][32 * h : 32 * (h + 1), 128 * c : 128 * (c + 1)],
                    rhs=qT[g][32 * h : 32 * (h + 1), :],
                    tile_position=(32 * h, 0),
                    start=True,
                    stop=True,
                )
            A = apool.tile([128, 2048], BF, tag="A", name="A")
            nc.scalar.activation(A[:], L[:], EXP)
            first, last = (c == 0), (c == k_chunks - 1)
            for h in range(4):
                fi = 32 * (4 * g + h)
                nc.tensor.matmul(
                    o_ps[32 * h : 32 * (h + 1), :],
                    lhsT=v_sb[:, 256 * c + fi : 256 * c + fi + 32],
                    rhs=A[:, 512 * h : 512 * (h + 1)],
                    tile_position=(0, 32 * h),
                    start=first,
                    stop=last,
                )
            for h in range(4):
                # all-ones [128, 32] lhsT replicates the softmax denominator
                # into every row of the head's 32-partition strip
                nc.tensor.matmul(
                    s_ps[32 * h : 32 * (h + 1), :],
                    lhsT=ones_b32[:, :],
                    rhs=A[:, 512 * h : 512 * (h + 1)],
                    tile_position=(0, 32 * h),
                    start=first,
                    stop=last,
                )
        # normalize: oT = o / s (s already strip-broadcast)
        r = apool.tile([128, 512], F32, tag="r", name="r")
        nc.vector.reciprocal(r[:], s_ps[:])
        oT = apool.tile([128, 512], BF, tag=f"oT{gtag}{g}", name=f"oT{g}")
        nc.vector.tensor_mul(oT[:], o_ps[:], r[:])
        oTs.append(oT)
    return oTs


def build_nc():
    nc = bass.Bass(num_devices=N_CORES)

    # ---- I/O declarations ----
    def din(name, shape, dt=BF):
        return nc.declare_dram_parameter(name, list(shape), dt, isOutput=False)

    xs = {m: din(f"x{m}", (E, 1024)) for m in "dle"}
    xq = din("xq", (E, Q))
    W = {}
    for m in "dle":
        for w in ("wq", "wk", "wv", "wo"):
            W[f"{w}_{m}"] = din(f"{w}_{m}", (E, E))
        W[f"bq_{m}"] = din(f"bq_{m}", (E, 1), F32)
        W[f"bk_{m}"] = din(f"bk_{m}", (E, 1), F32)
    bo_sum = din("bo_sum", (E, 1), F32)
    wg = din("wg", (E, NH))
    bg_row = din("bg_row", (1, NH))
    weT = din("weT", (NH * E, E))
    be_row = din("be_row", (1, NH * E))
    for w in ("wq", "wk", "wv", "wo"):
        W[f"{w}_f"] = din(f"{w}_f", (E, E))
    for b in ("bq_f", "bk_f", "bo_f"):
        W[b] = din(b, (E, 1), F32)
    OUT = nc.declare_dram_parameter("out", [E, Q], F32, isOutput=True)

    with tile.TileContext(nc) as tc, ExitStack() as top:
        wpool = top.enter_context(tc.tile_pool(name="w", bufs=1))
        xpool = top.enter_context(tc.tile_pool(name="x", bufs=1))
        spool = top.enter_context(tc.tile_pool(name="s", bufs=1))
        apool = top.enter_context(tc.tile_pool(name="a", bufs=2))
        dram = top.enter_context(tc.tile_pool(name="dram", bufs=1, space="DRAM"))

        # ---- constants ----
        ones_b32 = wpool.tile([128, 32], BF, tag="ones_b32", name="ones_b32")
        nc.vector.memset(ones_b32[:], 1.0)
        ones_row = wpool.tile([1, 128], BF, tag="ones_row", name="ones_row")
        nc.vector.memset(ones_row[:], 1.0)

        # ---- load weights ----
        wt = {}
        for m in "dle":
            for w in ("wq", "wk", "wv", "wo"):
                wt[f"{w}_{m}"] = _load_weight_pair(nc, wpool, f"{w}_{m}", W[f"{w}_{m}"], BF, E)
            for b in ("bq", "bk"):
                wt[f"{b}_{m}"] = _load_weight_pair(nc, wpool, f"{b}_{m}", W[f"{b}_{m}"], F32, 1)
        wt["bo_sum"] = _load_weight_pair(nc, wpool, "bo_sum", bo_sum, F32, 1)
        wt["wg"] = _load_weight_pair(nc, wpool, "wg", wg, BF, NH)
        bg_t = wpool.tile([1, NH], BF, tag="bg", name="bg_t")
        nc.sync.dma_start(out=bg_t[:], in_=bg_row[:])
        we_t = []
        for i in range(16):
            t = wpool.tile([128, E], BF, tag=f"we{i}", name=f"we{i}")
            nc.sync.dma_start(out=t[:], in_=weT[128 * i : 128 * (i + 1), :])
            we_t.append(t)
        be_t = wpool.tile([1, NH * E], BF, tag="be", name="be_t")
        nc.sync.dma_start(out=be_t[:], in_=be_row[:])
        for w in ("wq", "wk", "wv", "wo"):
            wt[f"{w}_f"] = _load_weight_pair(nc, wpool, f"{w}_f", W[f"{w}_f"], BF, E)
        for b in ("bq_f", "bk_f", "bo_f"):
            wt[b] = _load_weight_pair(nc, wpool, b, W[b], F32, 1)

        # ---- load activations ----
        x_t = {}
        for m in "dle":
            x_t[m] = _load_weight_pair(nc, xpool, f"x{m}", xs[m], BF, 1024)
        xq_t = _load_weight_pair(nc, xpool, "xq", xq, BF, Q)

        ag_in = dram.tile([4 * 128, E], BF, tag="ag_in", name="ag_in")
        ag_out = dram.tile(
            [N_CORES * 4 * 128, E], BF, addr_space="Shared", tag="ag_out", name="ag_out"
        )

        # ================= Phase A =================
        qkv = {}
        with tc.tile_pool(name="pp", bufs=2, space="PSUM") as pp:
            for m in "dle":
                qkv[f"q_{m}"] = _proj_fm(nc, pp, spool, f"qT_{m}", wt[f"wq_{m}"], xq_t, Q, bias=wt[f"bq_{m}"])
                qkv[f"k_{m}"] = _proj_fm(nc, pp, spool, f"kT_{m}", wt[f"wk_{m}"], x_t[m], 1024, bias=wt[f"bk_{m}"])
                qkv[f"v_{m}"] = _proj_tm(nc, pp, spool, f"v_{m}", wt[f"wv_{m}"], x_t[m], 1024)

        fused_sb = []
        with tc.tile_pool(name="fus", bufs=1, space="PSUM") as fusp:
            fused_ps = [fusp.tile([128, 512], F32, tag=f"fus{oc}", name=f"fus{oc}") for oc in range(2)]
            with tc.tile_pool(name="Lp", bufs=1, space="PSUM") as Lp, \
                 tc.tile_pool(name="osp", bufs=1, space="PSUM") as osp:
                for mi, m in enumerate("dle"):
                    oT = _emit_attention(
                        nc, Lp, osp, apool,
                        qkv[f"q_{m}"], qkv[f"k_{m}"], qkv[f"v_{m}"],
                        8, ones_b32, gtag=m,
                    )
                    for oc in range(2):
                        for g in range(2):
                            nc.tensor.matmul(
                                fused_ps[oc][:],
                                lhsT=wt[f"wo_{m}"][g][:, 128 * oc : 128 * (oc + 1)],
                                rhs=oT[g][:],
                                start=(mi == 0 and g == 0),
                                stop=(mi == 2 and g == 1),
                            )
            for oc in range(2):
                f = spool.tile([128, 512], BF, tag=f"fused{oc}", name=f"fused{oc}")
                nc.vector.tensor_scalar_add(f[:], fused_ps[oc][:], wt["bo_sum"][oc][:, 0:1])
                fused_sb.append(f)

        # ---- dense soft-MoE (token-major) ----
        with tc.tile_pool(name="mp", bufs=2, space="PSUM") as mp:
            gsb = []
            for tcn in range(4):
                gps = mp.tile([128, NH], F32, tag="g", name="g_ps")
                for ic in range(2):
                    nc.tensor.matmul(
                        gps[:],
                        lhsT=fused_sb[ic][:, 128 * tcn : 128 * (tcn + 1)],
                        rhs=wt["wg"][ic][:],
                        start=(ic == 0),
                        stop=False,
                    )
                nc.tensor.matmul(
                    gps[:], lhsT=ones_row[0:1, :], rhs=bg_t[0:1, :], start=False, stop=True
                )
                eg = apool.tile([128, NH], F32, tag="eg", name="eg")
                nc.scalar.activation(eg[:], gps[:], EXP)
                sg = apool.tile([128, 1], F32, tag="sg", name="sg")
                nc.vector.tensor_reduce(sg[:], eg[:], axis=mybir.AxisListType.X, op=mybir.AluOpType.add)
                rg = apool.tile([128, 1], F32, tag="rg", name="rg")
                nc.vector.reciprocal(rg[:], sg[:])
                g_n = spool.tile([128, NH], F32, tag=f"gn{tcn}", name=f"gn{tcn}")
                nc.vector.tensor_scalar_mul(g_n[:], eg[:], rg[:, 0:1])
                gsb.append(g_n)

            for tcn in range(4):
                macc = spool.tile([128, E], F32, tag=f"macc{tcn}", name=f"macc{tcn}")
                for e in range(NH):
                    yps = mp.tile([128, E], F32, tag="y", name="y_ps")
                    for ic in range(2):
                        nc.tensor.matmul(
                            yps[:],
                            lhsT=fused_sb[ic][:, 128 * tcn : 128 * (tcn + 1)],
                            rhs=we_t[2 * e + ic][:],
                            start=(ic == 0),
                            stop=False,
                        )
                    nc.tensor.matmul(
                        yps[:],
                        lhsT=ones_row[0:1, :],
                        rhs=be_t[0:1, E * e : E * (e + 1)],
                        start=False,
                        stop=True,
                    )
                    if e == 0:
                        nc.vector.tensor_scalar_mul(macc[:], yps[:], gsb[tcn][:, 0:1])
                    else:
                        yt = apool.tile([128, E], F32, tag="yt", name="yt")
                        nc.vector.tensor_scalar_mul(yt[:], yps[:], gsb[tcn][:, e : e + 1])
                        nc.vector.tensor_add(macc[:], macc[:], yt[:])
                mo = apool.tile([128, E], BF, tag="mo", name="mo")
                nc.vector.tensor_copy(out=mo[:], in_=macc[:])
                nc.sync.dma_start(out=ag_in[128 * tcn : 128 * (tcn + 1), :], in_=mo[:])

        # ================= exchange =================
        nc.gpsimd.collective_compute(
            "AllGather",
            mybir.AluOpType.bypass,
            replica_groups=[list(range(N_CORES))],
            ins=[ag_in[:].opt()],
            outs=[ag_out[:].opt()],
        )

        # ================= Phase B =================
        x2T, moeT = [], []
        for fc in range(2):
            t = spool.tile([128, 4096], BF, tag=f"x2T{fc}", name=f"x2T{fc}")
            nc.sync.dma_start_transpose(out=t[:], in_=ag_out[:, 128 * fc : 128 * (fc + 1)])
            x2T.append(t)
            t2 = spool.tile([128, Q], BF, tag=f"moeT{fc}", name=f"moeT{fc}")
            nc.sync.dma_start_transpose(out=t2[:], in_=ag_in[:, 128 * fc : 128 * (fc + 1)])
            moeT.append(t2)

        with tc.tile_pool(name="pp2", bufs=2, space="PSUM") as pp2:
            q2 = _proj_fm(nc, pp2, spool, "q2T", wt["wq_f"], moeT, Q, bias=wt["bq_f"])
            k2 = _proj_fm(nc, pp2, spool, "k2T", wt["wk_f"], x2T, 4096, bias=wt["bk_f"])
            v2 = _proj_tm(nc, pp2, spool, "v2", wt["wv_f"], x2T, 4096)

        with tc.tile_pool(name="Lp2", bufs=1, space="PSUM") as Lp2, \
             tc.tile_pool(name="osp2", bufs=1, space="PSUM") as osp2:
            oT = _emit_attention(nc, Lp2, osp2, apool, q2, k2, v2, 32, ones_b32, gtag="f")

        with tc.tile_pool(name="outp", bufs=1, space="PSUM") as outp:
            for oc in range(2):
                ops = outp.tile([128, 512], F32, tag=f"out{oc}", name=f"out{oc}")
                for g in range(2):
                    nc.tensor.matmul(
                        ops[:],
                        lhsT=wt["wo_f"][g][:, 128 * oc : 128 * (oc + 1)],
                        rhs=oT[g][:],
                        start=(g == 0),
                        stop=(g == 1),
                    )
                osb = apool.tile([128, 512], F32, tag="osb", name="osb")
                nc.vector.tensor_scalar_add(osb[:], ops[:], wt["bo_f"][oc][:, 0:1])
                nc.sync.dma_start(out=OUT[128 * oc : 128 * (oc + 1), :], in_=osb[:])

    _split_multi_waits(nc)
    return nc


# ------------------------------------------------------------------
# Host side
# ------------------------------------------------------------------

def _prep_maps(inputs):
    f32 = lambda a: np.ascontiguousarray(np.asarray(a, dtype=np.float32))
    bf = lambda a: np.ascontiguousarray(np.asarray(a).astype(BF_NP))
    s32 = math.sqrt(DH)

    imgs = {m: f32(inputs[n])[0] for m, n in (("d", "B_depth"), ("l", "B_lidar"), ("e", "B_event"))}

    shared = {}
    for m in "dle":
        Wi, bi = f32(inputs[f"Wi_{m}"]), f32(inputs[f"bi_{m}"])
        Wo, bo = f32(inputs[f"Wo_{m}"]), f32(inputs[f"bo_{m}"])
        shared[f"wq_{m}"] = bf((Wi[:E] / (3.0 * s32)).T)
        shared[f"bq_{m}"] = f32(bi[:E] / s32).reshape(E, 1)
        shared[f"wk_{m}"] = bf(Wi[E : 2 * E].T)
        shared[f"bk_{m}"] = f32(bi[E : 2 * E]).reshape(E, 1)
        shared[f"wv_{m}"] = bf(Wi[2 * E :].T)
        shared[f"wo_{m}"] = bf(Wo.T)
    bo_sum = np.zeros(E, np.float32)
    for m in "dle":
        Wi, bi = f32(inputs[f"Wi_{m}"]), f32(inputs[f"bi_{m}"])
        Wo, bo = f32(inputs[f"Wo_{m}"]), f32(inputs[f"bo_{m}"])
        bo_sum += bo + Wo @ bi[2 * E :]
    shared["bo_sum"] = bo_sum.reshape(E, 1)

    shared["wg"] = bf(f32(inputs["Wg"]).T)
    shared["bg_row"] = bf(f32(inputs["bg"]).reshape(1, NH))
    shared["weT"] = bf(f32(inputs["We"]).transpose(0, 2, 1).reshape(NH * E, E))
    shared["be_row"] = bf(f32(inputs["be"]).reshape(1, NH * E))

    Wi, bi = f32(inputs["Wi_m"]), f32(inputs["bi_m"])
    Wo, bo = f32(inputs["Wo_m"]), f32(inputs["bo_m"])
    shared["wq_f"] = bf((Wi[:E] / s32).T)
    shared["bq_f"] = f32(bi[:E] / s32).reshape(E, 1)
    shared["wk_f"] = bf(Wi[E : 2 * E].T)
    shared["bk_f"] = f32(bi[E : 2 * E]).reshape(E, 1)
    shared["wv_f"] = bf(Wi[2 * E :].T)
    shared["wo_f"] = bf(Wo.T)
    shared["bo_f"] = f32(bo + Wo @ bi[2 * E :]).reshape(E, 1)

    in_maps = []
    for c in range(N_CORES):
        b, h2 = c // 2, c % 2
        hb, wb = b // 2, b % 2
        blk = {
            m: imgs[m][:, 32 * hb : 32 * (hb + 1), 32 * wb : 32 * (wb + 1)].reshape(E, 1024)
            for m in "dle"
        }
        xsum = blk["d"] + blk["l"] + blk["e"]
        im = dict(shared)
        for m in "dle":
            im[f"x{m}"] = bf(blk[m])
        im["xq"] = bf(xsum[:, Q * h2 : Q * (h2 + 1)])
        in_maps.append(im)
    return in_maps


_NC_CACHE = {}


def _get_nc():
    if "nc" not in _NC_CACHE:
        _NC_CACHE["nc"] = build_nc()
    return _NC_CACHE["nc"]


def _assemble(results):
    out = np.zeros((1, E, 64, 64), np.float32)
    for c in range(N_CORES):
        b, h2 = c // 2, c % 2
        hb, wb = b // 2, b % 2
        o = results[c]["out"].reshape(E, 16, 32)
        out[0, :, 32 * hb + 16 * h2 : 32 * hb + 16 * (h2 + 1), 32 * wb : 32 * (wb + 1)] = o
    return out


def kernel(**inputs):
    nc = _get_nc()
    in_maps = _prep_maps(inputs)
    res = run_bass_kernel_spmd(nc, in_maps, core_ids=list(range(N_CORES)))
    return _assemble(res.results)
